# revision 1
# baseline (speedup 1.0000x reference)
"""GRU-style segmented-scan aggregator for Trainium2 (8 NeuronCores).

Reference computes, per node n with messages m_1..m_L sorted by time t:
    h <- W @ (m + h) + b   starting from h = 0
and returns the final h per node (zeros for empty nodes).

Because every step uses the SAME matrix W, the final state has the closed
form (h_0 = 0):
    h = sum_{k=0}^{L-1} W^{k+1} m_{(L-1-k)}  +  S_L b,   S_L = sum_{p<L} W^p
i.e. the k-th message FROM THE END is hit by W^{k+1}.  This turns the
sequential scan into independent batched matmuls against precomputed powers
of W -- ideal for the PE array.

Device layout (per core, SPMD over 8 cores):
  - nodes are sorted by message count (desc) and dealt round-robin to cores;
    each core owns <=1024 node slots, columns of a [256 feat x 1024] H^T
    accumulator kept in PSUM (2 chunks of 128 partitions).
  - step k multiplies W^{k+1} (lhsT, 4 chunks of 128x128) with the k-th-from-
    end messages of the first n_k slots (rhs, features on partitions), and
    accumulates into PSUM.  n_k shrinks as shorter segments are exhausted, so
    there is ~0% padding.
  - the bias term S_L b per node is added when copying PSUM -> SBUF -> HBM.

Host does the (cheap) data marshalling: lexsort by (index, t), gather into
the k-major column layout, precompute W powers in fp64, scatter results back.
"""

import numpy as np

import concourse.bass as bass
import concourse.mybir as mybir
from concourse import tile
from concourse.bass_utils import run_bass_kernel_spmd
import bass_rust

_N_PROCS = 27


class _SplitDrainTC(tile.TileContext):
    """TileContext whose kernel-tail drain is split into one drain per proc.

    The walrus build in this container rejects instructions carrying more
    than one sync wait; the stock tail drain waits on every proc at once.
    Emitting a chain of drains, each waiting on a single semaphore, is
    semantically identical (all procs quiesced before the exit barrier).
    """

    def _drain_and_barrier(self, tick_clock, wait_clock):
        gc = tick_clock.global_clock
        for p in range(_N_PROCS):
            if gc[p] <= 0:
                continue
            d = self.nc.sync.drain()
            vc = bass_rust.VectorClock(
                [gc[q] if q == p else 0 for q in range(_N_PROCS)])
            wait_clock.add_sem_waits(d.ins, bass_rust.ScopedClock({None: vc}))
        assert self.sems is not None
        popped = self.nc._tile_sem_poison_stack.pop()
        assert popped is self._sem_poison
        self.nc.all_engine_barrier()
        self.nc.clear_and_free_semaphores(list(self.sems.allocated().values()))
        self.nc.all_engine_barrier()

N_CORES = 8
DIM = 256
SLOTS = 1024  # node slots per core == PSUM accumulator width

_NC_CACHE: dict = {}


def _build_nc(K0: int, n_k: tuple, Cdev: int):
    """Build the Bass program for one core (shared by all 8 via SPMD).

    This walrus build accepts at most ONE sync wait per instruction, so the
    kernel is written with zero SBUF-slot reuse (every stream block gets its
    own tile; a reused slot would need WAR+WAW = 2 waits on its DMA) and the
    bias is injected via identity matmul instead of a DVE add (which would
    carry PE + DMA = 2 waits).
    """
    f32 = mybir.dt.float32
    nc = bass.Bass()

    # merged stream: per step k<K0, block = [512 weight cols | n_k hi | n_k lo];
    # final block = [128 identity cols | btermT chunk0 (1024) | chunk1 (1024)]
    Q = K0 * 512 + 2 * Cdev + 128 + 2 * SLOTS
    mw = nc.dram_tensor("mw", [128, Q], f32, kind="ExternalInput")
    out = nc.dram_tensor("out", [128, 2 * SLOTS], f32, kind="ExternalOutput")

    with _SplitDrainTC(nc) as tc:
        with (
            tc.tile_pool(name="m", bufs=1) as mpool,
            tc.tile_pool(name="misc", bufs=1) as miscpool,
            tc.tile_pool(name="ps", bufs=1, space="PSUM") as pspool,
        ):
            phs = [pspool.tile([128, SLOTS], f32, tag=f"ph{i}", name=f"ph{i}")
                   for i in range(2)]
            q = 0
            for k in range(K0):
                nk = n_k[k]
                blk = 512 + 2 * nk
                mk = mpool.tile([128, blk], f32, tag=f"mk{k}", name=f"mk{k}")
                nc.sync.dma_start(mk[:], mw[:, q:q + blk])
                for i in range(2):          # output feature chunk (PSUM partitions)
                    for j in range(2):      # contraction chunk
                        wt = mk[:, j * 256 + i * 128: j * 256 + (i + 1) * 128]
                        for s in range(0, nk, 512):
                            e = min(nk, s + 512)
                            nc.tensor.matmul(
                                phs[i][:, s:e], wt,
                                mk[:, 512 + j * nk + s: 512 + j * nk + e],
                                start=(k == 0 and j == 0),
                                stop=False,
                                skip_group_check=True,
                            )
                q += blk

            # bias (+ host-folded tail) via identity matmul: psum_i += I.T @ btT_i
            fb = mpool.tile([128, 128 + 2 * SLOTS], f32, tag="fb", name="fb")
            nc.sync.dma_start(fb[:], mw[:, q:q + 128 + 2 * SLOTS])
            for i in range(2):
                ident = fb[:, 0:128]
                for s in range(0, SLOTS, 512):
                    nc.tensor.matmul(
                        phs[i][:, s:s + 512], ident,
                        fb[:, 128 + i * SLOTS + s: 128 + i * SLOTS + s + 512],
                        start=False, stop=(s + 512 >= SLOTS),
                        skip_group_check=True,
                    )

            # writeback: PSUM -> SBUF copy (waits PE only), DMA out via SWDGE
            # (fresh DMASW lanes -> single producer wait)
            for i in range(2):
                ot = miscpool.tile([128, SLOTS], f32, tag=f"ot{i}", name=f"ot{i}")
                nc.vector.tensor_copy(ot[:], phs[i][:])
                nc.gpsimd.dma_start(out[:, i * SLOTS:(i + 1) * SLOTS], ot[:])
    return nc


def _prepare(msg, index, t, dim_size, W, b):
    """Host-side marshalling. Returns (in_maps, node_ids, schedule key)."""
    E, D = msg.shape
    counts = np.bincount(index, minlength=dim_size)
    order = np.lexsort((t, index))            # stable: primary index, secondary t
    msg_sorted = msg[order]                   # [E, D] grouped by node, t-ascending
    seg_starts = np.zeros(dim_size, np.int64)
    seg_starts[1:] = np.cumsum(counts)[:-1]

    nodesort = np.argsort(-counts, kind="stable")
    nz = nodesort[counts[nodesort] > 0]
    per_core = -(-len(nz) // N_CORES)
    assert per_core <= SLOTS, f"too many nodes per core: {per_core}"

    node_ids = np.full((N_CORES, SLOTS), -1, np.int64)
    for c in range(N_CORES):
        ids = nz[c::N_CORES]
        node_ids[c, :len(ids)] = ids
    cc = np.where(node_ids >= 0, counts[np.maximum(node_ids, 0)], 0)  # [8, SLOTS]

    Lmax = int(cc.max())
    n_k = tuple(int((cc > k).sum(axis=1).max()) for k in range(Lmax))

    # device handles steps k < K0; the tail (k >= K0, ~1-2% of messages) is
    # folded into the per-node bias term on the host.  K0 chosen so the
    # no-slot-reuse SBUF footprint fits (~<=176 KiB/partition).
    K0 = Lmax
    while K0 > 1 and (K0 * 512 + 2 * sum(n_k[:K0]) + 128 + 2 * SLOTS) > 44000:
        K0 -= 1
    Cdev = int(sum(n_k[:K0]))

    # column -> position in msg_sorted (or -1 = zero pad), k-major layout
    rowidx = np.full((N_CORES, Cdev), -1, np.int64)
    off = 0
    for k in range(K0):
        nk = n_k[k]
        nid = node_ids[:, :nk]
        ck = cc[:, :nk]
        active = k < ck
        pos = seg_starts[np.maximum(nid, 0)] + ck - 1 - k
        rowidx[:, off:off + nk] = np.where(active, pos, -1)
        off += nk

    # weights: powers of W in fp64, stored transposed (lhsT chunks).
    # wfull per k: cols [0,256) = (W^{k+1}).T rows 0:128 (j=0 chunk),
    #              cols [256,512) = rows 128:256 (j=1 chunk).
    Wd = W.astype(np.float64)
    bd = b.astype(np.float64)
    wfull = np.empty((128, K0 * 512), np.float32)
    s_table = np.zeros((Lmax + 1, D), np.float64)   # s_p = S_p b
    Wpows = []                                      # W^{k+1} (fp64), k = 0..Lmax-1
    P = Wd.copy()
    for k in range(Lmax):
        if k < K0:
            WT = P.T.astype(np.float32)             # (W^{k+1}).T
            wfull[:, k * 512:k * 512 + 256] = WT[:128, :]
            wfull[:, k * 512 + 256:(k + 1) * 512] = WT[128:, :]
        Wpows.append(P)
        s_table[k + 1] = Wd @ s_table[k] + bd
        P = P @ Wd

    # per-(core, slot) bias term: S_L b plus host-folded tail contributions
    bterm = s_table[cc]                              # [8, SLOTS, 256] fp64
    for k in range(K0, Lmax):
        nk = n_k[k]
        act = k < cc[:, :nk]                         # [8, nk]
        cs, ss = np.nonzero(act)
        pos = seg_starts[node_ids[cs, ss]] + cc[cs, ss] - 1 - k
        Y = msg_sorted[pos].astype(np.float64) @ Wpows[k].T
        bterm[cs, ss] += Y
    bterm32 = bterm.astype(np.float32)

    ident = np.zeros((128, 128), np.float32)
    np.fill_diagonal(ident, 1.0)

    Q = K0 * 512 + 2 * Cdev + 128 + 2 * SLOTS
    in_maps = []
    for c in range(N_CORES):
        ri = rowidx[c]
        Mg = msg_sorted[np.maximum(ri, 0)]
        Mg[ri < 0] = 0.0                             # [Cdev, 256]
        hi = Mg[:, :128].T                           # [128, Cdev]
        lo = Mg[:, 128:].T
        mwb = np.empty((128, Q), np.float32)
        off = 0
        q = 0
        for k in range(K0):
            nk = n_k[k]
            mwb[:, q:q + 512] = wfull[:, k * 512:(k + 1) * 512]
            mwb[:, q + 512:q + 512 + nk] = hi[:, off:off + nk]
            mwb[:, q + 512 + nk:q + 512 + 2 * nk] = lo[:, off:off + nk]
            off += nk
            q += 512 + 2 * nk
        mwb[:, q:q + 128] = ident
        mwb[:, q + 128:q + 128 + SLOTS] = bterm32[c, :, :128].T
        mwb[:, q + 128 + SLOTS:q + 128 + 2 * SLOTS] = bterm32[c, :, 128:].T
        in_maps.append({"mw": mwb})
    return in_maps, node_ids, (K0, n_k[:K0], Cdev)


def _run(inputs: dict, trace: bool = False, **run_kwargs):
    msg = np.ascontiguousarray(np.asarray(inputs["msg"], dtype=np.float32))
    index = np.asarray(inputs["index"]).astype(np.int64)
    t = np.asarray(inputs["t"], dtype=np.float32)
    W = np.asarray(inputs["W"], dtype=np.float32)
    b = np.asarray(inputs["b"], dtype=np.float32)
    dim_size = int(inputs["dim_size"])

    in_maps, node_ids, key = _prepare(msg, index, t, dim_size, W, b)
    K0, n_k, Cdev = key
    if key not in _NC_CACHE:
        _NC_CACHE[key] = _build_nc(K0, n_k, Cdev)
    nc = _NC_CACHE[key]

    res = run_bass_kernel_spmd(nc, in_maps, list(range(N_CORES)),
                               trace=trace, **run_kwargs)

    hidden = np.zeros((dim_size, DIM), np.float32)
    for c in range(N_CORES):
        o = res.results[c]["out"]                    # [128, 2*SLOTS]
        hc = np.concatenate([o[:, :SLOTS], o[:, SLOTS:]], axis=0).T  # [SLOTS, 256]
        valid = node_ids[c] >= 0
        hidden[node_ids[c][valid]] = hc[valid]
    return hidden, res


def kernel(**inputs) -> np.ndarray:
    hidden, _ = _run(inputs, trace=False)
    return hidden



# revision 3
# speedup vs baseline: 2.6914x; 2.6914x over previous
"""GRU-style segmented-scan aggregator for Trainium2 (8 NeuronCores).

Reference computes, per node n with messages m_1..m_L sorted by time t:
    h <- W @ (m + h) + b   starting from h = 0
and returns the final h per node (zeros for empty nodes).

Because every step uses the SAME matrix W, the final state has the closed
form (h_0 = 0):
    h = sum_{k=0}^{L-1} W^{k+1} m_{(L-1-k)}  +  S_L b,   S_L = sum_{p<L} W^p
i.e. the k-th message FROM THE END is hit by W^{k+1}.  This turns the
sequential scan into independent batched matmuls against precomputed powers
of W -- ideal for the PE array.

Device layout (per core, SPMD over 8 cores):
  - nodes are sorted by message count (desc) and dealt round-robin to cores;
    each core owns <=1024 node slots, columns of a [256 feat x 1024] H^T
    accumulator kept in PSUM (2 chunks of 128 partitions).
  - step k multiplies W^{k+1} (lhsT, 4 chunks of 128x128, bf16) with the
    k-th-from-end messages of the first n_k slots (rhs bf16, features on
    partitions), accumulating into fp32 PSUM.  n_k shrinks as shorter
    segments are exhausted, so there is ~0% padding.
  - blocks are emitted smallest-first (descending k): the tail blocks are
    weight-DMA-heavy and run while the PE is still ramping, so DMA gets
    ahead of the PE for the big head blocks.
  - the bias term S_L b per node and the deep tail (k >= K0, n_k below
    TAIL_NK) are folded into a host-side fp64 add after the gather.

Host does the (cheap) data marshalling: lexsort by (index, t), gather into
the k-major column layout, precompute W powers in fp64, scatter results back.
"""

import numpy as np
import ml_dtypes

import concourse.bass as bass
import concourse.mybir as mybir
from concourse import tile
from concourse.bass_utils import run_bass_kernel_spmd
import bass_rust

_N_PROCS = 27


class _SplitDrainTC(tile.TileContext):
    """TileContext whose kernel-tail drain is split into one drain per proc.

    The walrus build in this container rejects instructions carrying more
    than one sync wait; the stock tail drain waits on every proc at once.
    Emitting a chain of drains, each waiting on a single semaphore, is
    semantically identical (all procs quiesced before the exit barrier).
    """

    def _drain_and_barrier(self, tick_clock, wait_clock):
        gc = tick_clock.global_clock
        for p in range(_N_PROCS):
            if gc[p] <= 0:
                continue
            d = self.nc.sync.drain()
            vc = bass_rust.VectorClock(
                [gc[q] if q == p else 0 for q in range(_N_PROCS)])
            wait_clock.add_sem_waits(d.ins, bass_rust.ScopedClock({None: vc}))
        assert self.sems is not None
        popped = self.nc._tile_sem_poison_stack.pop()
        assert popped is self._sem_poison
        self.nc.all_engine_barrier()
        self.nc.clear_and_free_semaphores(list(self.sems.allocated().values()))
        self.nc.all_engine_barrier()

N_CORES = 8
DIM = 256
SLOTS = 1024   # node slots per core == PSUM accumulator width
TAIL_NK = 64   # steps with fewer active slots than this fold to the host
BF16 = ml_dtypes.bfloat16

_NC_CACHE: dict = {}


def _build_nc(K0: int, n_k: tuple, Cdev: int):
    """Build the Bass program for one core (shared by all 8 via SPMD).

    This walrus build accepts at most ONE sync wait per instruction, so the
    kernel is written with zero SBUF-slot reuse: every stream block gets its
    own tile (a reused slot would need WAR+WAW = 2 waits on its DMA).
    """
    f32 = mybir.dt.float32
    bf16 = mybir.dt.bfloat16
    nc = bass.Bass()

    # merged stream: per step k<K0, block = [512 weight cols | n_k hi | n_k lo]
    Q = K0 * 512 + 2 * Cdev
    mw = nc.dram_tensor("mw", [128, Q], bf16, kind="ExternalInput")
    out = nc.dram_tensor("out", [128, 2 * SLOTS], bf16, kind="ExternalOutput")

    offs = []
    q = 0
    for k in range(K0):
        offs.append(q)
        q += 512 + 2 * n_k[k]

    with _SplitDrainTC(nc) as tc:
        with (
            tc.tile_pool(name="m", bufs=1) as mpool,
            tc.tile_pool(name="misc", bufs=1) as miscpool,
            tc.tile_pool(name="ps", bufs=1, space="PSUM") as pspool,
        ):
            phs = [pspool.tile([128, SLOTS], f32, tag=f"ph{i}", name=f"ph{i}")
                   for i in range(2)]
            # start=True resets the whole PSUM bank, so the first block (k=0,
            # n_0 = SLOTS) must initialize every column; later blocks only
            # accumulate.  Column c's final writer is the last k with
            # n_k > c, which carries stop=True.
            for k in range(K0):
                nk = n_k[k]
                nk_next = n_k[k + 1] if k + 1 < K0 else 0
                blk = 512 + 2 * nk
                q0 = offs[k]
                mk = mpool.tile([128, blk], bf16, tag=f"mk{k}", name=f"mk{k}")
                nc.sync.dma_start(mk[:], mw[:, q0:q0 + blk])
                pts = sorted({0, nk, nk_next} | set(range(512, nk, 512)))
                pts = [p for p in pts if p <= nk]
                for i in range(2):          # output feature chunk (PSUM partitions)
                    for j in range(2):      # contraction chunk
                        wt = mk[:, j * 256 + i * 128: j * 256 + (i + 1) * 128]
                        for a, e in zip(pts[:-1], pts[1:]):
                            nc.tensor.matmul(
                                phs[i][:, a:e], wt,
                                mk[:, 512 + j * nk + a: 512 + j * nk + e],
                                start=(j == 0 and k == 0),
                                stop=(j == 1 and a >= nk_next),
                                skip_group_check=True,
                            )

            # writeback: PSUM -> SBUF bf16 copy (waits PE only), DMA out via
            # SWDGE (fresh DMASW lanes -> single producer wait)
            for i in range(2):
                ot = miscpool.tile([128, SLOTS], bf16, tag=f"ot{i}", name=f"ot{i}")
                nc.vector.tensor_copy(ot[:], phs[i][:])
                nc.gpsimd.dma_start(out[:, i * SLOTS:(i + 1) * SLOTS], ot[:])
    return nc


def _prepare(msg, index, t, dim_size, W, b):
    """Host-side marshalling. Returns (in_maps, node_ids, bterm, key)."""
    E, D = msg.shape
    counts = np.bincount(index, minlength=dim_size)
    order = np.lexsort((t, index))            # stable: primary index, secondary t
    msg_sorted = msg[order]                   # [E, D] grouped by node, t-ascending
    seg_starts = np.zeros(dim_size, np.int64)
    seg_starts[1:] = np.cumsum(counts)[:-1]

    nodesort = np.argsort(-counts, kind="stable")
    nz = nodesort[counts[nodesort] > 0]
    per_core = -(-len(nz) // N_CORES)
    assert per_core <= SLOTS, f"too many nodes per core: {per_core}"

    node_ids = np.full((N_CORES, SLOTS), -1, np.int64)
    for c in range(N_CORES):
        ids = nz[c::N_CORES]
        node_ids[c, :len(ids)] = ids
    cc = np.where(node_ids >= 0, counts[np.maximum(node_ids, 0)], 0)  # [8, SLOTS]

    Lmax = int(cc.max())
    n_k = tuple(int((cc > k).sum(axis=1).max()) for k in range(Lmax))

    # device handles steps k < K0; the deep tail (few active slots) is
    # folded into the per-node bias term on the host in fp64.
    K0 = Lmax
    while K0 > 1 and n_k[K0 - 1] < TAIL_NK:
        K0 -= 1
    while K0 > 1 and (K0 * 512 + 2 * sum(n_k[:K0])) * 2 > 190000:
        K0 -= 1            # SBUF budget (bytes/partition), never binds here
    Cdev = int(sum(n_k[:K0]))

    # column -> position in msg_sorted (or -1 = zero pad), k-major layout
    rowidx = np.full((N_CORES, Cdev), -1, np.int64)
    off = 0
    for k in range(K0):
        nk = n_k[k]
        nid = node_ids[:, :nk]
        ck = cc[:, :nk]
        active = k < ck
        pos = seg_starts[np.maximum(nid, 0)] + ck - 1 - k
        rowidx[:, off:off + nk] = np.where(active, pos, -1)
        off += nk

    # weights: powers of W in fp64, stored transposed (lhsT chunks).
    # wfull per k: cols [0,256) = (W^{k+1}).T rows 0:128 (j=0 chunk),
    #              cols [256,512) = rows 128:256 (j=1 chunk).
    Wd = W.astype(np.float64)
    bd = b.astype(np.float64)
    wfull = np.empty((128, K0 * 512), BF16)
    s_table = np.zeros((Lmax + 1, D), np.float64)   # s_p = S_p b
    Wpows = []                                      # W^{k+1} (fp64), k = 0..Lmax-1
    P = Wd.copy()
    for k in range(Lmax):
        if k < K0:
            WT = P.T.astype(np.float32)             # (W^{k+1}).T
            wfull[:, k * 512:k * 512 + 256] = WT[:128, :].astype(BF16)
            wfull[:, k * 512 + 256:(k + 1) * 512] = WT[128:, :].astype(BF16)
        Wpows.append(P)
        s_table[k + 1] = Wd @ s_table[k] + bd
        P = P @ Wd

    # per-(core, slot) bias term: S_L b plus host-folded tail contributions
    bterm = s_table[cc]                              # [8, SLOTS, 256] fp64
    for k in range(K0, Lmax):
        nk = n_k[k]
        act = k < cc[:, :nk]                         # [8, nk]
        cs, ss = np.nonzero(act)
        pos = seg_starts[node_ids[cs, ss]] + cc[cs, ss] - 1 - k
        Y = msg_sorted[pos].astype(np.float64) @ Wpows[k].T
        bterm[cs, ss] += Y
    bterm32 = bterm.astype(np.float32)

    Q = K0 * 512 + 2 * Cdev
    in_maps = []
    for c in range(N_CORES):
        ri = rowidx[c]
        Mg = msg_sorted[np.maximum(ri, 0)]
        Mg[ri < 0] = 0.0                             # [Cdev, 256]
        Mgb = Mg.astype(BF16)
        hi = Mgb[:, :128].T                          # [128, Cdev]
        lo = Mgb[:, 128:].T
        mwb = np.empty((128, Q), BF16)
        off = 0
        q = 0
        for k in range(K0):
            nk = n_k[k]
            mwb[:, q:q + 512] = wfull[:, k * 512:(k + 1) * 512]
            mwb[:, q + 512:q + 512 + nk] = hi[:, off:off + nk]
            mwb[:, q + 512 + nk:q + 512 + 2 * nk] = lo[:, off:off + nk]
            off += nk
            q += 512 + 2 * nk
        in_maps.append({"mw": mwb})
    return in_maps, node_ids, bterm32, (K0, n_k[:K0], Cdev)


def _run(inputs: dict, trace: bool = False, **run_kwargs):
    msg = np.ascontiguousarray(np.asarray(inputs["msg"], dtype=np.float32))
    index = np.asarray(inputs["index"]).astype(np.int64)
    t = np.asarray(inputs["t"], dtype=np.float32)
    W = np.asarray(inputs["W"], dtype=np.float32)
    b = np.asarray(inputs["b"], dtype=np.float32)
    dim_size = int(inputs["dim_size"])

    in_maps, node_ids, bterm32, key = _prepare(msg, index, t, dim_size, W, b)
    K0, n_k, Cdev = key
    if key not in _NC_CACHE:
        _NC_CACHE[key] = _build_nc(K0, n_k, Cdev)
    nc = _NC_CACHE[key]

    res = run_bass_kernel_spmd(nc, in_maps, list(range(N_CORES)),
                               trace=trace, **run_kwargs)

    hidden = np.zeros((dim_size, DIM), np.float32)
    for c in range(N_CORES):
        o = np.asarray(res.results[c]["out"]).astype(np.float32)  # [128, 2*SLOTS]
        hc = np.concatenate([o[:, :SLOTS], o[:, SLOTS:]], axis=0).T  # [SLOTS, 256]
        hc += bterm32[c]
        valid = node_ids[c] >= 0
        hidden[node_ids[c][valid]] = hc[valid]
    return hidden, res


def kernel(**inputs) -> np.ndarray:
    hidden, _ = _run(inputs, trace=False)
    return hidden


# revision 6
# speedup vs baseline: 3.3651x; 1.2503x over previous
"""GRU-style segmented-scan aggregator for Trainium2 (8 NeuronCores).

Reference computes, per node n with messages m_1..m_L sorted by time t:
    h <- W @ (m + h) + b   starting from h = 0
and returns the final h per node (zeros for empty nodes).

Because every step uses the SAME matrix W, the final state has the closed
form (h_0 = 0):
    h = sum_{k=0}^{L-1} W^{k+1} m_{(L-1-k)}  +  S_L b,   S_L = sum_{p<L} W^p
i.e. the k-th message FROM THE END is hit by W^{k+1}.  This turns the
sequential scan into independent batched matmuls against precomputed powers
of W -- ideal for the PE array.

Device layout (per core, SPMD over 8 cores):
  - nodes are sorted by message count (desc) and dealt round-robin to cores;
    each core owns <=1024 node slots, columns of a [256 feat x 1024] H^T
    accumulator kept in PSUM (2x2 tiles of 128 part x 512).
  - step k multiplies W^{k+1} (lhsT, 4 chunks of 128x128, bf16) with the
    k-th-from-end messages of the first n_k slots (rhs bf16, features on
    partitions), accumulating into fp32 PSUM.  n_k shrinks as shorter
    segments are exhausted, so there is ~0% padding.
  - a dozen dependency-free warm-up matmuls on junk SBUF run during the
    NEFF entry ceremony so the PE clock (HAM) is ramped before real data
    lands; block 0's stream is split into two single-producer tiles so its
    j=0 matmuls start after half the transfer.
  - PSUM columns [512,1024) take their last write at the final block with
    n_k > 512, so that half is cast+written back overlapped with the
    remaining matmuls; only columns [0,512) drain at the end (DVE and
    GpSimd each cast one tile in parallel).
  - the bias term S_L b per node and the deep tail (k >= K0, n_k below
    TAIL_NK) are folded into a host-side add after the gather.

Host does the (cheap) data marshalling: lexsort by (index, t), gather into
the k-major column layout, precompute W powers in fp64, scatter results back.
"""

import numpy as np
import ml_dtypes

import concourse.bass as bass
import concourse.mybir as mybir
from concourse import tile
from concourse.bass_utils import run_bass_kernel_spmd
import bass_rust

_N_PROCS = 27


class _SplitDrainTC(tile.TileContext):
    """TileContext whose kernel-tail drain is split into one drain per proc.

    The walrus build in this container rejects instructions carrying more
    than one sync wait; the stock tail drain waits on every proc at once.
    Emitting a chain of drains, each waiting on a single semaphore, is
    semantically identical (all procs quiesced before the exit barrier).
    """

    def _drain_and_barrier(self, tick_clock, wait_clock):
        gc = tick_clock.global_clock
        for p in range(_N_PROCS):
            if gc[p] <= 0:
                continue
            d = self.nc.sync.drain()
            vc = bass_rust.VectorClock(
                [gc[q] if q == p else 0 for q in range(_N_PROCS)])
            wait_clock.add_sem_waits(d.ins, bass_rust.ScopedClock({None: vc}))
        assert self.sems is not None
        popped = self.nc._tile_sem_poison_stack.pop()
        assert popped is self._sem_poison
        self.nc.all_engine_barrier()
        self.nc.clear_and_free_semaphores(list(self.sems.allocated().values()))
        self.nc.all_engine_barrier()

N_CORES = 8
DIM = 256
SLOTS = 1024    # node slots per core == PSUM accumulator width
HALF = 512      # one PSUM bank of fp32
TAIL_NK = 200   # steps with fewer active slots than this fold to the host
WARMUP_MMS = 12
BF16 = ml_dtypes.bfloat16

_NC_CACHE: dict = {}


def _build_nc(K0: int, n_k: tuple, Cdev: int):
    """Build the Bass program for one core (shared by all 8 via SPMD).

    This walrus build accepts at most ONE sync wait per instruction, so the
    kernel is written with zero SBUF-slot reuse: every stream block gets its
    own tile (a reused slot would need WAR+WAW = 2 waits on its DMA), and
    each matmul's weights and rhs always live in the same tile (a second
    producer would add a second wait).
    """
    f32 = mybir.dt.float32
    bf16 = mybir.dt.bfloat16
    nc = bass.Bass()

    # stream: block 0 = [256 w_j0 | n_0 hi] [256 w_j1 | n_0 lo];
    # blocks k>=1 = [512 w | n_k hi | n_k lo]
    Q = K0 * 512 + 2 * Cdev
    mw = nc.dram_tensor("mw", [128, Q], bf16, kind="ExternalInput")
    out = nc.dram_tensor("out", [128, 2 * SLOTS], bf16, kind="ExternalOutput")

    offs = []
    q = 0
    for k in range(K0):
        offs.append(q)
        q += 512 + 2 * n_k[k]

    # last block still writing PSUM columns [HALF, SLOTS)
    k_hi = max((k for k in range(K0) if n_k[k] > HALF), default=0)

    with _SplitDrainTC(nc) as tc:
        with (
            tc.tile_pool(name="m", bufs=1) as mpool,
            tc.tile_pool(name="misc", bufs=1) as miscpool,
            tc.tile_pool(name="ps", bufs=1, space="PSUM") as pspool,
        ):
            # phs[i][h]: output features [i*128,(i+1)*128) x slots [h*512,...)
            phs = [[pspool.tile([128, HALF], f32, tag=f"ph{i}{h}",
                                name=f"ph{i}{h}") for h in range(2)]
                   for i in range(2)]

            # PE warm-up on junk SBUF (no producers -> no waits): ramps the
            # HAM clock during the NEFF entry ceremony / first DMA.
            wsrc = mpool.tile([128, 128 + HALF], bf16, tag="wsrc", name="wsrc")
            wps = pspool.tile([128, HALF], f32, tag="wps", name="wps")
            nc.vector.memset(wsrc[:], 0.0)
            for _ in range(WARMUP_MMS):
                nc.tensor.matmul(wps[:], wsrc[:, 0:128], wsrc[:, 128:128 + HALF],
                                 start=True, stop=True, skip_group_check=True)

            def emit_mms(k, j, i, wt, rhs_base, rhs_off):
                """Matmuls of block k, contraction chunk j, feature chunk i.

                rhs columns a (absolute slot index) live at
                rhs_base[:, rhs_off + a].
                """
                nk = n_k[k]
                nk_next = n_k[k + 1] if k + 1 < K0 else 0
                pts = sorted({0, nk, nk_next, HALF} | set())
                pts = [p for p in pts if p <= nk]
                if pts[0] != 0:
                    pts = [0] + pts
                for a, e in zip(pts[:-1], pts[1:]):
                    h = a // HALF
                    ph = phs[i][h]
                    nc.tensor.matmul(
                        ph[:, a - h * HALF: e - h * HALF], wt,
                        rhs_base[:, rhs_off + a: rhs_off + e],
                        start=(j == 0 and k == 0),
                        stop=(j == 1 and a >= nk_next),
                        skip_group_check=True,
                    )

            emitted_hi_cast = False
            for k in range(K0):
                nk = n_k[k]
                q0 = offs[k]
                if k == 0:
                    # two single-producer tiles so j=0 can start after the
                    # first half of the transfer
                    half = 256 + nk
                    ta = mpool.tile([128, half], bf16, tag="mk0a", name="mk0a")
                    tb = mpool.tile([128, half], bf16, tag="mk0b", name="mk0b")
                    nc.sync.dma_start(ta[:], mw[:, q0:q0 + half])
                    nc.sync.dma_start(tb[:], mw[:, q0 + half:q0 + 2 * half])
                    for j, tl in ((0, ta), (1, tb)):
                        for i in range(2):
                            emit_mms(k, j, i, tl[:, i * 128:(i + 1) * 128],
                                     tl, 256)
                else:
                    blk = 512 + 2 * nk
                    mk = mpool.tile([128, blk], bf16, tag=f"mk{k}",
                                    name=f"mk{k}")
                    nc.sync.dma_start(mk[:], mw[:, q0:q0 + blk])
                    for i in range(2):
                        for j in range(2):
                            emit_mms(k, j, i,
                                     mk[:, j * 256 + i * 128:
                                        j * 256 + (i + 1) * 128],
                                     mk, 512 + j * nk)

                if k == k_hi and not emitted_hi_cast:
                    emitted_hi_cast = True
                    # columns [HALF, SLOTS) are final: drain them now,
                    # overlapped with the remaining blocks' matmuls.
                    # out DMAs ride SWDGE so they don't block later input
                    # DMAs in the sync HWDGE FIFO.
                    for i in range(2):
                        ot = miscpool.tile([128, HALF], bf16,
                                           tag=f"ot{i}1", name=f"ot{i}1")
                        if i == 0:
                            nc.vector.tensor_copy(ot[:], phs[i][1][:])
                        else:
                            nc.scalar.copy(ot[:], phs[i][1][:])
                        nc.gpsimd.dma_start(
                            out[:, i * SLOTS + HALF:(i + 1) * SLOTS], ot[:])

            # final writeback of columns [0, HALF): one cast on DVE, one on
            # the Activation engine, in parallel
            for i in range(2):
                ot = miscpool.tile([128, HALF], bf16, tag=f"ot{i}0",
                                   name=f"ot{i}0")
                if i == 0:
                    nc.vector.tensor_copy(ot[:], phs[i][0][:])
                else:
                    nc.scalar.copy(ot[:], phs[i][0][:])
                nc.gpsimd.dma_start(out[:, i * SLOTS:i * SLOTS + HALF], ot[:])
    return nc


def _prepare(msg, index, t, dim_size, W, b):
    """Host-side marshalling. Returns (in_maps, node_ids, bterm, key)."""
    E, D = msg.shape
    counts = np.bincount(index, minlength=dim_size)
    order = np.lexsort((t, index))            # stable: primary index, secondary t
    msg_sorted = msg[order]                   # [E, D] grouped by node, t-ascending
    seg_starts = np.zeros(dim_size, np.int64)
    seg_starts[1:] = np.cumsum(counts)[:-1]

    nodesort = np.argsort(-counts, kind="stable")
    nz = nodesort[counts[nodesort] > 0]
    per_core = -(-len(nz) // N_CORES)
    assert per_core <= SLOTS, f"too many nodes per core: {per_core}"

    node_ids = np.full((N_CORES, SLOTS), -1, np.int64)
    for c in range(N_CORES):
        ids = nz[c::N_CORES]
        node_ids[c, :len(ids)] = ids
    cc = np.where(node_ids >= 0, counts[np.maximum(node_ids, 0)], 0)  # [8, SLOTS]

    Lmax = int(cc.max())
    n_k = tuple(int((cc > k).sum(axis=1).max()) for k in range(Lmax))

    # device handles steps k < K0; the deep tail (few active slots) is
    # folded into the per-node bias term on the host in fp64.
    K0 = Lmax
    while K0 > 1 and n_k[K0 - 1] < TAIL_NK:
        K0 -= 1
    while K0 > 1 and (K0 * 512 + 2 * sum(n_k[:K0])) * 2 > 190000:
        K0 -= 1            # SBUF budget (bytes/partition), never binds here
    Cdev = int(sum(n_k[:K0]))

    # column -> position in msg_sorted (or -1 = zero pad), k-major layout
    rowidx = np.full((N_CORES, Cdev), -1, np.int64)
    off = 0
    for k in range(K0):
        nk = n_k[k]
        nid = node_ids[:, :nk]
        ck = cc[:, :nk]
        active = k < ck
        pos = seg_starts[np.maximum(nid, 0)] + ck - 1 - k
        rowidx[:, off:off + nk] = np.where(active, pos, -1)
        off += nk

    # weights: powers of W in fp64, stored transposed (lhsT chunks).
    # wfull per k: cols [0,256) = (W^{k+1}).T rows 0:128 (j=0 chunk),
    #              cols [256,512) = rows 128:256 (j=1 chunk).
    Wd = W.astype(np.float64)
    bd = b.astype(np.float64)
    wfull = np.empty((128, K0 * 512), BF16)
    s_table = np.zeros((Lmax + 1, D), np.float64)   # s_p = S_p b
    Wpows = []                                      # W^{k+1} (fp64), k = 0..Lmax-1
    P = Wd.copy()
    for k in range(Lmax):
        if k < K0:
            WT = P.T.astype(np.float32)             # (W^{k+1}).T
            wfull[:, k * 512:k * 512 + 256] = WT[:128, :].astype(BF16)
            wfull[:, k * 512 + 256:(k + 1) * 512] = WT[128:, :].astype(BF16)
        Wpows.append(P)
        s_table[k + 1] = Wd @ s_table[k] + bd
        P = P @ Wd

    # per-(core, slot) bias term: S_L b plus host-folded tail contributions
    bterm = s_table[cc]                              # [8, SLOTS, 256] fp64
    for k in range(K0, Lmax):
        nk = n_k[k]
        act = k < cc[:, :nk]                         # [8, nk]
        cs, ss = np.nonzero(act)
        pos = seg_starts[node_ids[cs, ss]] + cc[cs, ss] - 1 - k
        Y = msg_sorted[pos].astype(np.float64) @ Wpows[k].T
        bterm[cs, ss] += Y
    bterm32 = bterm.astype(np.float32)

    Q = K0 * 512 + 2 * Cdev
    in_maps = []
    for c in range(N_CORES):
        ri = rowidx[c]
        Mg = msg_sorted[np.maximum(ri, 0)]
        Mg[ri < 0] = 0.0                             # [Cdev, 256]
        Mgb = Mg.astype(BF16)
        hi = Mgb[:, :128].T                          # [128, Cdev]
        lo = Mgb[:, 128:].T
        mwb = np.empty((128, Q), BF16)
        off = 0
        q = 0
        for k in range(K0):
            nk = n_k[k]
            if k == 0:
                # [256 w_j0 | hi] [256 w_j1 | lo]
                mwb[:, q:q + 256] = wfull[:, 0:256]
                mwb[:, q + 256:q + 256 + nk] = hi[:, :nk]
                q2 = q + 256 + nk
                mwb[:, q2:q2 + 256] = wfull[:, 256:512]
                mwb[:, q2 + 256:q2 + 256 + nk] = lo[:, :nk]
            else:
                mwb[:, q:q + 512] = wfull[:, k * 512:(k + 1) * 512]
                mwb[:, q + 512:q + 512 + nk] = hi[:, off:off + nk]
                mwb[:, q + 512 + nk:q + 512 + 2 * nk] = lo[:, off:off + nk]
            off += nk
            q += 512 + 2 * nk
        in_maps.append({"mw": mwb})
    return in_maps, node_ids, bterm32, (K0, n_k[:K0], Cdev)


def _run(inputs: dict, trace: bool = False, **run_kwargs):
    msg = np.ascontiguousarray(np.asarray(inputs["msg"], dtype=np.float32))
    index = np.asarray(inputs["index"]).astype(np.int64)
    t = np.asarray(inputs["t"], dtype=np.float32)
    W = np.asarray(inputs["W"], dtype=np.float32)
    b = np.asarray(inputs["b"], dtype=np.float32)
    dim_size = int(inputs["dim_size"])

    in_maps, node_ids, bterm32, key = _prepare(msg, index, t, dim_size, W, b)
    K0, n_k, Cdev = key
    if key not in _NC_CACHE:
        _NC_CACHE[key] = _build_nc(K0, n_k, Cdev)
    nc = _NC_CACHE[key]

    res = run_bass_kernel_spmd(nc, in_maps, list(range(N_CORES)),
                               trace=trace, **run_kwargs)

    hidden = np.zeros((dim_size, DIM), np.float32)
    for c in range(N_CORES):
        o = np.asarray(res.results[c]["out"]).astype(np.float32)  # [128, 2*SLOTS]
        hc = np.concatenate([o[:, :SLOTS], o[:, SLOTS:]], axis=0).T  # [SLOTS, 256]
        hc += bterm32[c]
        valid = node_ids[c] >= 0
        hidden[node_ids[c][valid]] = hc[valid]
    return hidden, res


def kernel(**inputs) -> np.ndarray:
    hidden, _ = _run(inputs, trace=False)
    return hidden


# revision 10
# speedup vs baseline: 3.4411x; 1.0226x over previous
"""GRU-style segmented-scan aggregator for Trainium2 (8 NeuronCores).

Reference computes, per node n with messages m_1..m_L sorted by time t:
    h <- W @ (m + h) + b   starting from h = 0
and returns the final h per node (zeros for empty nodes).

Because every step uses the SAME matrix W, the final state has the closed
form (h_0 = 0):
    h = sum_{k=0}^{L-1} W^{k+1} m_{(L-1-k)}  +  S_L b,   S_L = sum_{p<L} W^p
i.e. the k-th message FROM THE END is hit by W^{k+1}.  This turns the
sequential scan into independent batched matmuls against precomputed powers
of W -- ideal for the PE array.

Device layout (per core, SPMD over 8 cores):
  - nodes are sorted by message count (desc) and dealt round-robin to cores;
    each core owns <=1024 node slots, columns of a [256 feat x 1024] H^T
    accumulator kept in PSUM (2x2 tiles of 128 part x 512).
  - step k multiplies W^{k+1} (lhsT, 4 chunks of 128x128, bf16) with the
    k-th-from-end messages of the first n_k slots (rhs bf16, features on
    partitions), accumulating into fp32 PSUM.  n_k shrinks as shorter
    segments are exhausted, so there is ~0% padding.
  - a dozen dependency-free warm-up matmuls on junk SBUF run during the
    NEFF entry ceremony so the PE clock (HAM) is ramped before real data
    lands; block 0's stream is split into two single-producer tiles so its
    j=0 matmuls start after half the transfer.
  - PSUM columns [512,1024) take their last write at the final block with
    n_k > 512, so that half is cast+written back overlapped with the
    remaining matmuls; only columns [0,512) drain at the end (DVE and
    GpSimd each cast one tile in parallel).
  - the bias term S_L b per node and the deep tail (k >= K0, n_k below
    TAIL_NK) are folded into a host-side add after the gather.

Host does the (cheap) data marshalling: lexsort by (index, t), gather into
the k-major column layout, precompute W powers in fp64, scatter results back.
"""

import numpy as np
import ml_dtypes

import concourse.bass as bass
import concourse.mybir as mybir
from concourse import tile
from concourse.bass_utils import run_bass_kernel_spmd
import bass_rust

_N_PROCS = 27


class _SplitDrainTC(tile.TileContext):
    """TileContext whose kernel-tail drain is split into one drain per proc.

    The walrus build in this container rejects instructions carrying more
    than one sync wait; the stock tail drain waits on every proc at once.
    Emitting a chain of drains, each waiting on a single semaphore, is
    semantically identical (all procs quiesced before the exit barrier).
    """

    def _drain_and_barrier(self, tick_clock, wait_clock):
        gc = tick_clock.global_clock
        for p in range(_N_PROCS):
            if gc[p] <= 0:
                continue
            d = self.nc.sync.drain()
            vc = bass_rust.VectorClock(
                [gc[q] if q == p else 0 for q in range(_N_PROCS)])
            wait_clock.add_sem_waits(d.ins, bass_rust.ScopedClock({None: vc}))
        assert self.sems is not None
        popped = self.nc._tile_sem_poison_stack.pop()
        assert popped is self._sem_poison
        self.nc.all_engine_barrier()
        self.nc.clear_and_free_semaphores(list(self.sems.allocated().values()))
        self.nc.all_engine_barrier()

N_CORES = 8
DIM = 256
SLOTS = 1024    # node slots per core == PSUM accumulator width
HALF = 512      # one PSUM bank of fp32
TAIL_NK = 300   # steps with fewer active slots than this fold to the host
WARMUP_MMS = 6
BF16 = ml_dtypes.bfloat16

_NC_CACHE: dict = {}


def _build_nc(K0: int, n_k: tuple, Cdev: int):
    """Build the Bass program for one core (shared by all 8 via SPMD).

    This walrus build accepts at most ONE sync wait per instruction, so the
    kernel is written with zero SBUF-slot reuse: every stream block gets its
    own tile (a reused slot would need WAR+WAW = 2 waits on its DMA), and
    each matmul's weights and rhs always live in the same tile (a second
    producer would add a second wait).
    """
    f32 = mybir.dt.float32
    bf16 = mybir.dt.bfloat16
    nc = bass.Bass()

    # stream: block 0 = [256 w_j0 | n_0 hi] [256 w_j1 | n_0 lo];
    # blocks k>=1 = [512 w | n_k hi | n_k lo]
    Q = K0 * 512 + 2 * Cdev
    mw = nc.dram_tensor("mw", [128, Q], bf16, kind="ExternalInput")
    # one output tensor per PSUM quadrant: avoids cross-queue WAW tracking
    # (a shared out tensor would give the HWDGE out-DMA a second sync wait)
    outs = [[nc.dram_tensor(f"out{i}{h}", [128, HALF], bf16,
                            kind="ExternalOutput") for h in range(2)]
            for i in range(2)]

    offs = []
    q = 0
    for k in range(K0):
        offs.append(q)
        q += 512 + 2 * n_k[k]

    # last block still writing PSUM columns [HALF, SLOTS)
    k_hi = max((k for k in range(K0) if n_k[k] > HALF), default=0)

    with _SplitDrainTC(nc) as tc:
        with (
            tc.tile_pool(name="m", bufs=1) as mpool,
            tc.tile_pool(name="misc", bufs=1) as miscpool,
            tc.tile_pool(name="ps", bufs=1, space="PSUM") as pspool,
        ):
            # phs[i][h]: output features [i*128,(i+1)*128) x slots [h*512,...)
            phs = [[pspool.tile([128, HALF], f32, tag=f"ph{i}{h}",
                                name=f"ph{i}{h}") for h in range(2)]
                   for i in range(2)]

            # PE warm-up on junk SBUF (no producers -> no waits): ramps the
            # HAM clock during the NEFF entry ceremony / first DMA.
            wsrc = mpool.tile([128, 128 + HALF], bf16, tag="wsrc", name="wsrc")
            wps = pspool.tile([128, HALF], f32, tag="wps", name="wps")
            nc.vector.memset(wsrc[:], 0.0)
            for _ in range(WARMUP_MMS):
                nc.tensor.matmul(wps[:], wsrc[:, 0:128], wsrc[:, 128:128 + HALF],
                                 start=True, stop=True, skip_group_check=True)

            def emit_mms(k, j, i, wt, rhs_base, rhs_off):
                """Matmuls of block k, contraction chunk j, feature chunk i.

                rhs columns a (absolute slot index) live at
                rhs_base[:, rhs_off + a].
                """
                nk = n_k[k]
                pts = [p for p in (0, HALF, nk) if p <= nk]
                if pts[-1] != nk:
                    pts.append(nk)
                for a, e in zip(pts[:-1], pts[1:]):
                    h = a // HALF
                    ph = phs[i][h]
                    nc.tensor.matmul(
                        ph[:, a - h * HALF: e - h * HALF], wt,
                        rhs_base[:, rhs_off + a: rhs_off + e],
                        start=(j == 0 and k == 0),
                        stop=(j == 1 and e == nk),
                        skip_group_check=True,
                    )

            emitted_hi_cast = False
            for k in range(K0):
                nk = n_k[k]
                q0 = offs[k]
                if k == 0:
                    # two single-producer tiles so j=0 can start after the
                    # first half of the transfer
                    half = 256 + nk
                    ta = mpool.tile([128, half], bf16, tag="mk0a", name="mk0a")
                    tb = mpool.tile([128, half], bf16, tag="mk0b", name="mk0b")
                    nc.sync.dma_start(ta[:], mw[:, q0:q0 + half])
                    nc.sync.dma_start(tb[:], mw[:, q0 + half:q0 + 2 * half])
                    for j, tl in ((0, ta), (1, tb)):
                        for i in range(2):
                            emit_mms(k, j, i, tl[:, i * 128:(i + 1) * 128],
                                     tl, 256)
                else:
                    blk = 512 + 2 * nk
                    mk = mpool.tile([128, blk], bf16, tag=f"mk{k}",
                                    name=f"mk{k}")
                    nc.sync.dma_start(mk[:], mw[:, q0:q0 + blk])
                    for i in range(2):
                        for j in range(2):
                            emit_mms(k, j, i,
                                     mk[:, j * 256 + i * 128:
                                        j * 256 + (i + 1) * 128],
                                     mk, 512 + j * nk)

                if k == k_hi and not emitted_hi_cast:
                    emitted_hi_cast = True
                    # columns [HALF, SLOTS) are final: drain them now,
                    # overlapped with the remaining blocks' matmuls.
                    # out DMAs ride SWDGE so they don't block later input
                    # DMAs in the sync HWDGE FIFO.
                    for i in range(2):
                        ot = miscpool.tile([128, HALF], bf16,
                                           tag=f"ot{i}1", name=f"ot{i}1")
                        if i == 0:
                            nc.vector.tensor_copy(ot[:], phs[i][1][:])
                            nc.gpsimd.dma_start(outs[i][1][:], ot[:])
                        else:
                            nc.scalar.copy(ot[:], phs[i][1][:])
                            nc.gpsimd.dma_start(outs[i][1][:], ot[:])

            # final writeback of columns [0, HALF): one cast on DVE, one on
            # the Activation engine; the two out DMAs ride different DGE
            # queues (sync HWDGE is idle by now) so they overlap too
            for i in range(2):
                ot = miscpool.tile([128, HALF], bf16, tag=f"ot{i}0",
                                   name=f"ot{i}0")
                if i == 0:
                    nc.vector.tensor_copy(ot[:], phs[i][0][:])
                    nc.gpsimd.dma_start(outs[i][0][:], ot[:])
                else:
                    nc.scalar.copy(ot[:], phs[i][0][:])
                    nc.gpsimd.dma_start(outs[i][0][:], ot[:])
    return nc


def _prepare(msg, index, t, dim_size, W, b):
    """Host-side marshalling. Returns (in_maps, node_ids, bterm, key)."""
    E, D = msg.shape
    counts = np.bincount(index, minlength=dim_size)
    order = np.lexsort((t, index))            # stable: primary index, secondary t
    msg_sorted = msg[order]                   # [E, D] grouped by node, t-ascending
    seg_starts = np.zeros(dim_size, np.int64)
    seg_starts[1:] = np.cumsum(counts)[:-1]

    nodesort = np.argsort(-counts, kind="stable")
    nz = nodesort[counts[nodesort] > 0]
    per_core = -(-len(nz) // N_CORES)
    assert per_core <= SLOTS, f"too many nodes per core: {per_core}"

    node_ids = np.full((N_CORES, SLOTS), -1, np.int64)
    for c in range(N_CORES):
        ids = nz[c::N_CORES]
        node_ids[c, :len(ids)] = ids
    cc = np.where(node_ids >= 0, counts[np.maximum(node_ids, 0)], 0)  # [8, SLOTS]

    Lmax = int(cc.max())
    n_k = tuple(int((cc > k).sum(axis=1).max()) for k in range(Lmax))

    # device handles steps k < K0; the deep tail (few active slots) is
    # folded into the per-node bias term on the host in fp64.
    K0 = Lmax
    while K0 > 1 and n_k[K0 - 1] < TAIL_NK:
        K0 -= 1
    while K0 > 1 and (K0 * 512 + 2 * sum(n_k[:K0])) * 2 > 190000:
        K0 -= 1            # SBUF budget (bytes/partition), never binds here
    Cdev = int(sum(n_k[:K0]))

    # column -> position in msg_sorted (or -1 = zero pad), k-major layout
    rowidx = np.full((N_CORES, Cdev), -1, np.int64)
    off = 0
    for k in range(K0):
        nk = n_k[k]
        nid = node_ids[:, :nk]
        ck = cc[:, :nk]
        active = k < ck
        pos = seg_starts[np.maximum(nid, 0)] + ck - 1 - k
        rowidx[:, off:off + nk] = np.where(active, pos, -1)
        off += nk

    # weights: powers of W in fp64, stored transposed (lhsT chunks).
    # wfull per k: cols [0,256) = (W^{k+1}).T rows 0:128 (j=0 chunk),
    #              cols [256,512) = rows 128:256 (j=1 chunk).
    Wd = W.astype(np.float64)
    bd = b.astype(np.float64)
    wfull = np.empty((128, K0 * 512), BF16)
    s_table = np.zeros((Lmax + 1, D), np.float64)   # s_p = S_p b
    Wpows = []                                      # W^{k+1} (fp64), k = 0..Lmax-1
    P = Wd.copy()
    for k in range(Lmax):
        if k < K0:
            WT = P.T.astype(np.float32)             # (W^{k+1}).T
            wfull[:, k * 512:k * 512 + 256] = WT[:128, :].astype(BF16)
            wfull[:, k * 512 + 256:(k + 1) * 512] = WT[128:, :].astype(BF16)
        Wpows.append(P)
        s_table[k + 1] = Wd @ s_table[k] + bd
        P = P @ Wd

    # per-(core, slot) bias term: S_L b plus host-folded tail contributions
    bterm = s_table[cc]                              # [8, SLOTS, 256] fp64
    for k in range(K0, Lmax):
        nk = n_k[k]
        act = k < cc[:, :nk]                         # [8, nk]
        cs, ss = np.nonzero(act)
        pos = seg_starts[node_ids[cs, ss]] + cc[cs, ss] - 1 - k
        Y = msg_sorted[pos].astype(np.float64) @ Wpows[k].T
        bterm[cs, ss] += Y
    bterm32 = bterm.astype(np.float32)

    Q = K0 * 512 + 2 * Cdev
    in_maps = []
    for c in range(N_CORES):
        ri = rowidx[c]
        Mg = msg_sorted[np.maximum(ri, 0)]
        Mg[ri < 0] = 0.0                             # [Cdev, 256]
        Mgb = Mg.astype(BF16)
        hi = Mgb[:, :128].T                          # [128, Cdev]
        lo = Mgb[:, 128:].T
        mwb = np.empty((128, Q), BF16)
        off = 0
        q = 0
        for k in range(K0):
            nk = n_k[k]
            if k == 0:
                # [256 w_j0 | hi] [256 w_j1 | lo]
                mwb[:, q:q + 256] = wfull[:, 0:256]
                mwb[:, q + 256:q + 256 + nk] = hi[:, :nk]
                q2 = q + 256 + nk
                mwb[:, q2:q2 + 256] = wfull[:, 256:512]
                mwb[:, q2 + 256:q2 + 256 + nk] = lo[:, :nk]
            else:
                mwb[:, q:q + 512] = wfull[:, k * 512:(k + 1) * 512]
                mwb[:, q + 512:q + 512 + nk] = hi[:, off:off + nk]
                mwb[:, q + 512 + nk:q + 512 + 2 * nk] = lo[:, off:off + nk]
            off += nk
            q += 512 + 2 * nk
        in_maps.append({"mw": mwb})
    return in_maps, node_ids, bterm32, (K0, n_k[:K0], Cdev)


def _run(inputs: dict, trace: bool = False, **run_kwargs):
    msg = np.ascontiguousarray(np.asarray(inputs["msg"], dtype=np.float32))
    index = np.asarray(inputs["index"]).astype(np.int64)
    t = np.asarray(inputs["t"], dtype=np.float32)
    W = np.asarray(inputs["W"], dtype=np.float32)
    b = np.asarray(inputs["b"], dtype=np.float32)
    dim_size = int(inputs["dim_size"])

    in_maps, node_ids, bterm32, key = _prepare(msg, index, t, dim_size, W, b)
    K0, n_k, Cdev = key
    if key not in _NC_CACHE:
        _NC_CACHE[key] = _build_nc(K0, n_k, Cdev)
    nc = _NC_CACHE[key]

    res = run_bass_kernel_spmd(nc, in_maps, list(range(N_CORES)),
                               trace=trace, **run_kwargs)

    hidden = np.zeros((dim_size, DIM), np.float32)
    for c in range(N_CORES):
        r = res.results[c]
        o = np.concatenate(
            [np.concatenate([np.asarray(r[f"out{i}{h}"]) for h in range(2)],
                            axis=1) for i in range(2)],
            axis=0).astype(np.float32)                   # [256, SLOTS]
        hc = o.T                                          # [SLOTS, 256]
        hc += bterm32[c]
        valid = node_ids[c] >= 0
        hidden[node_ids[c][valid]] = hc[valid]
    return hidden, res


def kernel(**inputs) -> np.ndarray:
    hidden, _ = _run(inputs, trace=False)
    return hidden
